# revision 1
# baseline (speedup 1.0000x reference)
"""Trainium2 Bass kernel for nn_CGLayer (gnn_message_passing).

Contract: kernel(**inputs) takes FULL inputs (as reference.setup_inputs()),
returns FULL output [8,128,1,16,9] f32. Internally: data-parallel over the
batch dim across 8 NeuronCores; per core one batch element.

Algebraic reduction (exact):
  X   = conn @ vertices                  (message passing, per batch)
  Y   = mix_nl(cg(X, X))                 (per-node quadratic in X)
  S   = sum_j sph[:, j, :]               (neighbor sum commutes through the
  Z   = mix_rel(cg(Y, S))                 relative-CG stage: x-side is
  out = Z / sqrt(sum Z^2 / 16)            j-independent)

Device pipeline per core (features-on-free "i-partition" layout for
products; PE transposes flip to slot-partition layout for the big mix):
  A:  X[i,144]    = matmul(lhsT=connT, rhs=vcat)
  S:  Ssum[i,9]   = reduce_j(sph);  S[9,i] = PE-transpose
  B:  P[i,17920]  = 14 stride-0 DVE tensor_tensor ops (pair products,
                    s-group padded slot layout; W2 host-combined CG x w_nl)
      PT chunks   = 140 PE transposes + PSUM->SBUF copies
      Y[144,i]    = 140 matmuls lhsT=W2-chunks (PSUM-accumulated per s-group)
  C:  P2          = Y (piece layout) * replicated S rows (DVE)
      Z[144,i]    = 36 matmuls lhsT=W3-chunks
Host epilogue: gather, unpack e=(l,c',k), global normalization per l.
"""
import numpy as np
from math import factorial, sqrt

MAXL = 2
CH = 16
NN = 128
NB = 8
LDIM = [1, 3, 5]
FOFF = [0, 16, 64]
NF = 144
TAU_NL = [768, 1536, 1536]
TAU_REL = [48, 96, 96]

# ------------------------------------------------------------- CG tables
def _cg_coeff(j1, m1, j2, m2, j3, m3):
    if m3 != m1 + m2:
        return 0.0
    pre = sqrt((2 * j3 + 1) * factorial(j3 + j1 - j2) * factorial(j3 - j1 + j2)
               * factorial(j1 + j2 - j3) / factorial(j1 + j2 + j3 + 1))
    pre *= sqrt(factorial(j3 + m3) * factorial(j3 - m3) * factorial(j1 - m1)
                * factorial(j1 + m1) * factorial(j2 - m2) * factorial(j2 + m2))
    s = 0.0
    vmin = max(0, j2 - j3 - m1, j1 - j3 + m2)
    vmax = min(j1 + j2 - j3, j1 - m1, j2 + m2)
    for v in range(vmin, vmax + 1):
        s += (-1) ** v / (factorial(v) * factorial(j1 + j2 - j3 - v)
                          * factorial(j1 - m1 - v) * factorial(j2 + m2 - v)
                          * factorial(j3 - j2 + m1 + v) * factorial(j3 - j1 - m2 + v))
    return pre * s


def _cg_matrix(l1, l2, l):
    M = np.zeros((2 * l1 + 1, 2 * l2 + 1, 2 * l + 1))
    for m1 in range(-l1, l1 + 1):
        for m2 in range(-l2, l2 + 1):
            if -l <= m1 + m2 <= l:
                M[m1 + l1, m2 + l2, m1 + m2 + l] = _cg_coeff(l1, m1, l2, m2, l, m1 + m2)
    return M


def _valid_pairs(l):
    return [(l1, l2) for l1 in range(3) for l2 in range(3)
            if abs(l1 - l2) <= l <= l1 + l2]

# ----------------------------------------------------- stage-B slot layout
Q_COMBOS = [(l1, l2, m1) for l1 in range(3) for l2 in range(l1, 3)
            for m1 in range(2 * l1 + 1)]
NQ = len(Q_COMBOS)                       # 14
GRP = NQ * 256                           # 3584
NSLOT = 5 * GRP                          # 17920
NCHUNK = NSLOT // 128                    # 140
GCH = GRP // 128                         # 28 chunks per s-group


def _sgroup_cols(g):
    st = g - 2
    return [(l, cp) for l in range(3) if abs(st) <= l for cp in range(CH)]

SG_NCOL = [len(_sgroup_cols(g)) for g in range(5)]      # [16,32,48,32,16]
YOFF = np.concatenate([[0], np.cumsum(SG_NCOL)])        # piece row offsets


def _sg_lblock_col(g, l):
    st = g - 2
    return 16 * sum(1 for lp in range(l) if abs(st) <= lp)


def _product_ops():
    ops = []
    for qi, (l1, l2, m1) in enumerate(Q_COMBOS):
        mt1 = m1 - l1
        m2_lo = max(0, -2 - mt1 + l2)
        m2_hi = min(2 * l2, 2 - mt1 + l2)
        n_m2 = m2_hi - m2_lo + 1
        g0 = mt1 + (m2_lo - l2) + 2
        ops.append(dict(l1=l1, l2=l2, m1=m1, m2_lo=m2_lo, n_m2=n_m2,
                        out_off=g0 * GRP + qi * 256))
    return ops

PRODUCT_OPS = _product_ops()
_QIDX = {q: i for i, q in enumerate(Q_COMBOS)}
_CAR, _DAR = np.meshgrid(np.arange(16), np.arange(16), indexing="ij")


def _assemble_W2(w_nl):
    """W2[NSLOT, 48] f64->f32: combined CG x w_nl, s-grouped slot layout."""
    W2 = np.zeros((NSLOT, 48))
    for l in range(3):
        off = 0
        for (p1, p2) in _valid_pairs(l):
            Cg = _cg_matrix(p1, p2, l)
            wl = np.asarray(w_nl[l], np.float64)
            for m1 in range(2 * p1 + 1):
                for m2 in range(2 * p2 + 1):
                    st = (m1 - p1) + (m2 - p2)
                    if abs(st) > l:
                        continue
                    gc = Cg[m1, m2, st + l]
                    if gc == 0.0:
                        continue
                    g = st + 2
                    if p1 <= p2:
                        slots = g * GRP + _QIDX[(p1, p2, m1)] * 256 + _CAR * 16 + _DAR
                    else:
                        slots = g * GRP + _QIDX[(p2, p1, m2)] * 256 + _DAR * 16 + _CAR
                    t = off + _CAR * 16 + _DAR
                    c0 = _sg_lblock_col(g, l)
                    W2[slots.ravel(), c0:c0 + 16] += gc * wl[t.ravel(), :]
            off += 256
    return W2.astype(np.float32)


def _assemble_W3(w_rel):
    """W3[9*144, 144]: contraction P2[(n,a), i] -> Z[e, i]; a = Y piece row."""
    SOFF = [0, 1, 4]
    W3 = np.zeros((9 * 144, 144))
    ar = np.arange(16)
    for l in range(3):
        off = 0
        for (p1, p2) in _valid_pairs(l):
            Cg = _cg_matrix(p1, p2, l)
            wr = np.asarray(w_rel[l], np.float64)
            for m1 in range(2 * p1 + 1):
                for m2 in range(2 * p2 + 1):
                    st = (m1 - p1) + (m2 - p2)
                    if abs(st) > l:
                        continue
                    gc = Cg[m1, m2, st + l]
                    if gc == 0.0:
                        continue
                    gY = (m1 - p1) + 2
                    a0 = YOFF[gY] + _sg_lblock_col(gY, p1)
                    rows = (SOFF[p2] + m2) * 144 + a0 + ar
                    cols = FOFF[l] + (st + l) + ar * LDIM[l]
                    W3[np.ix_(rows, cols)] += gc * wr[off:off + 16, :]
            off += 16
    return W3.astype(np.float32)

# ------------------------------------------------------------ bass builder
_NC_CACHE = {}


def _build_nc(debug=False):
    import concourse.bacc as bacc
    import concourse.bass as bass
    import concourse.tile as tile
    from concourse import mybir
    from concourse.masks import make_identity

    f32 = mybir.dt.float32
    nc = bacc.Bacc()
    d_connT = nc.declare_dram_parameter("connT", [128, 128], f32, isOutput=False)
    d_vcat = nc.declare_dram_parameter("vcat", [128, NF], f32, isOutput=False)
    d_sph = nc.declare_dram_parameter("sph", [128, 128 * 9], f32, isOutput=False)
    d_w2 = nc.declare_dram_parameter("w2", [128, NCHUNK * 48], f32, isOutput=False)
    d_w3g = nc.declare_dram_parameter("w3g", [48, 9 * 5 * 144], f32, isOutput=False)
    d_zout = nc.declare_dram_parameter("zout", [144, 128], f32, isOutput=True)
    if debug:
        d_dbgx = nc.declare_dram_parameter("dbgx", [128, NF], f32, isOutput=True)
        d_dbgs = nc.declare_dram_parameter("dbgs", [9, 128], f32, isOutput=True)
        d_dbgp = nc.declare_dram_parameter("dbgp", [128, NSLOT], f32, isOutput=True)
        d_dbgy = nc.declare_dram_parameter("dbgy", [48, 5 * 128], f32, isOutput=True)
        d_dbgr = nc.declare_dram_parameter("dbgr", [48, 128], f32, isOutput=True)
        d_dbgp2 = nc.declare_dram_parameter("dbgp2", [48, 5 * 128], f32, isOutput=True)

    def vap(t, doff, freedims):
        base = t[:] if not isinstance(t, bass.AP) else t
        return bass.AP(tensor=base.tensor, offset=base.offset + doff,
                       ap=[list(base.ap[0])] + [list(d) for d in freedims])

    with tile.TileContext(nc) as tc:
      with (
        tc.tile_pool(name="big", bufs=1) as big,
        tc.tile_pool(name="sb", bufs=1) as sb,
        tc.tile_pool(name="pt", bufs=4) as ptp,
        tc.tile_pool(name="p2", bufs=4) as p2p,
        tc.tile_pool(name="ps_x", bufs=1, space="PSUM") as ps_x,
        tc.tile_pool(name="ps_t", bufs=2, space="PSUM") as ps_t,
        tc.tile_pool(name="ps_y", bufs=1, space="PSUM") as ps_y,
        tc.tile_pool(name="ps_r", bufs=1, space="PSUM") as ps_r,
        tc.tile_pool(name="ps_z", bufs=1, space="PSUM") as ps_z,
      ):
        # ---- input DMAs
        connT = sb.tile([128, 128], f32)
        nc.sync.dma_start(out=connT, in_=d_connT[:, :])
        vcat = sb.tile([128, NF], f32)
        nc.sync.dma_start(out=vcat, in_=d_vcat[:, :])
        sph = big.tile([128, 128 * 9], f32)
        for q in range(4):
            nc.sync.dma_start(out=sph[:, q * 288:(q + 1) * 288],
                              in_=d_sph[:, q * 288:(q + 1) * 288])
        w2 = big.tile([128, NCHUNK, 48], f32)
        for q in range(4):
            s = q * (NCHUNK // 4) * 48
            e = (q + 1) * (NCHUNK // 4) * 48
            nc.sync.dma_start(out=vap(w2, s, [[1, e - s]]),
                              in_=d_w2[:, s:e])
        w3g = sb.tile([48, 9, 5, 144], f32)
        nc.sync.dma_start(
            out=w3g, in_=d_w3g[:, :].rearrange("p (n g e) -> p n g e", n=9, g=5))
        ident = sb.tile([128, 128], f32)
        make_identity(nc, ident)

        # ---- stage A: X[i, feat] = connT.T @ vcat
        x_ps = ps_x.tile([128, NF], f32, tag="misc", name="x_ps")
        nc.tensor.matmul(x_ps, connT, vcat, start=True, stop=True)
        X = sb.tile([128, NF], f32)
        nc.scalar.activation(X, x_ps, mybir.ActivationFunctionType.Copy)

        # ---- stage S: Ssum[i, 9] = sum_j sph; S[9, i]
        ssum = sb.tile([128, 9], f32)
        nc.vector.tensor_reduce(
            ssum, vap(sph, 0, [[1, 9], [9, 128]]),
            mybir.AxisListType.X, mybir.AluOpType.add)
        s_ps = ps_x.tile([128, NF], f32, tag="misc", name="s_ps")[0:9, 0:128]
        nc.tensor.transpose(s_ps, ssum, ident)
        S = sb.tile([9, 128], f32)
        nc.scalar.activation(S, s_ps, mybir.ActivationFunctionType.Copy)
        sel9 = sb.tile([9, 9, 128], f32)
        nc.gpsimd.memset(sel9, 0.0)
        nc.gpsimd.affine_select(
            out=sel9, in_=sel9, compare_op=mybir.AluOpType.not_equal,
            fill=1.0, base=0, pattern=[[-1, 9], [0, 128]], channel_multiplier=1)

        # ---- stage B products: P[i, NSLOT]
        P = big.tile([128, NSLOT], f32)
        for h in range(2):  # memset split across two engines
            eng = nc.gpsimd if h == 0 else nc.vector
            eng.memset(P[:, h * (NSLOT // 2):(h + 1) * (NSLOT // 2)], 0.0)
        for op in PRODUCT_OPS:
            l1, l2, m1 = op["l1"], op["l2"], op["m1"]
            nm2 = op["n_m2"]
            nc.vector.tensor_tensor(
                out=vap(P, op["out_off"], [[GRP, nm2], [16, 16], [1, 16]]),
                in0=vap(X, FOFF[l1] + m1, [[0, nm2], [LDIM[l1], 16], [0, 16]]),
                in1=vap(X, FOFF[l2] + op["m2_lo"],
                        [[1, nm2], [0, 16], [LDIM[l2], 16]]),
                op=mybir.AluOpType.mult)

        # ---- stage B transposes + mix: Y pieces, PSUM-accumulated
        ymix = ps_y.tile([48, 5, 128], f32)
        cp_engines = [nc.scalar, nc.vector]
        for g in range(5):
            ncol = SG_NCOL[g]
            for ch in range(GCH):
                t_ps = ps_t.tile([128, 128], f32)
                nc.tensor.transpose(
                    t_ps, P[:, (g * GCH + ch) * 128:(g * GCH + ch + 1) * 128], ident)
                pt = ptp.tile([128, 128], f32)
                eng = cp_engines[(g * GCH + ch) % 2]
                if eng is nc.scalar:
                    nc.scalar.activation(pt, t_ps, mybir.ActivationFunctionType.Copy)
                else:
                    eng.tensor_copy(out=pt, in_=t_ps)
                nc.tensor.matmul(ymix[0:ncol, g, :], w2[:, g * GCH + ch, 0:ncol], pt,
                                 start=(ch == 0), stop=(ch == GCH - 1))

        # Y pieces -> SBUF, packed [48, 5, 128]
        ysb = sb.tile([48, 5, 128], f32)
        nc.scalar.activation(ysb, ymix, mybir.ActivationFunctionType.Copy)

        if debug:
            nc.sync.dma_start(out=d_dbgx[:, :], in_=X)
            nc.sync.dma_start(out=d_dbgs[:, :], in_=S)
            for q in range(4):
                nc.sync.dma_start(out=d_dbgp[:, q * 4480:(q + 1) * 4480],
                                  in_=P[:, q * 4480:(q + 1) * 4480])
            nc.sync.dma_start(out=d_dbgy[:, :], in_=ysb)

        # ---- stage C: P2 = Y * rep(S_n); Z = sum_n W3_n.T @ P2_n
        z_hi = ps_z.tile([128, 128], f32)
        z_lo = ps_z.tile([16, 128], f32)
        for n in range(9):
            rep = ps_r.tile([48, 128], f32)
            nc.tensor.matmul(rep, sel9[:, n, 0:48], S, start=True, stop=True)
            p2 = p2p.tile([48, 5, 128], f32)
            nc.vector.tensor_tensor(
                out=p2,
                in0=vap(rep, 0, [[0, 5], [1, 128]]),
                in1=ysb, op=mybir.AluOpType.mult)
            if debug and n == 0:
                rep_sb = sb.tile([48, 128], f32)
                nc.vector.tensor_copy(out=rep_sb, in_=rep)
                nc.sync.dma_start(out=d_dbgr[:, :], in_=rep_sb)
                nc.sync.dma_start(out=d_dbgp2[:, :], in_=p2)
            for g in range(5):
                ncol = SG_NCOL[g]
                nc.tensor.matmul(z_hi, w3g[0:ncol, n, g, 0:128], p2[0:ncol, g, :],
                                 start=(n == 0 and g == 0), stop=(n == 8 and g == 4))
                nc.tensor.matmul(z_lo, w3g[0:ncol, n, g, 128:144], p2[0:ncol, g, :],
                                 start=(n == 0 and g == 0), stop=(n == 8 and g == 4))

        zs_hi = sb.tile([128, 128], f32)
        zs_lo = sb.tile([16, 128], f32)
        nc.scalar.activation(zs_hi, z_hi, mybir.ActivationFunctionType.Copy)
        nc.scalar.activation(zs_lo, z_lo, mybir.ActivationFunctionType.Copy)
        nc.sync.dma_start(out=d_zout[0:128, :], in_=zs_hi)
        nc.sync.dma_start(out=d_zout[128:144, :], in_=zs_lo)

    nc.compile()
    return nc

# ------------------------------------------------------------- host entry
def _get_nc():
    if "nc" not in _NC_CACHE:
        _NC_CACHE["nc"] = _build_nc()
    return _NC_CACHE["nc"]


def kernel(vertices_0, vertices_1, vertices_2, connectivity,
           sph_0, sph_1, sph_2,
           w_nl_0, w_nl_1, w_nl_2,
           w_rel_0, w_rel_1, w_rel_2):
    from concourse.bass_utils import run_bass_kernel_spmd

    f = np.float32
    verts = [np.asarray(v, f) for v in (vertices_0, vertices_1, vertices_2)]
    sphs = [np.asarray(s, f) for s in (sph_0, sph_1, sph_2)]
    conn = np.asarray(connectivity)
    W2 = _assemble_W2([np.asarray(w, f) for w in (w_nl_0, w_nl_1, w_nl_2)])
    W3 = _assemble_W3([np.asarray(w, f) for w in (w_rel_0, w_rel_1, w_rel_2)])
    # pack to SBUF-ready layouts (shared across cores)
    w2p = np.ascontiguousarray(
        W2.reshape(NCHUNK, 128, 48).transpose(1, 0, 2).reshape(128, NCHUNK * 48))
    W3r = W3.reshape(9, 144, 144)
    w3g = np.zeros((48, 9, 5, 144), np.float32)
    for g in range(5):
        w3g[0:SG_NCOL[g], :, g, :] = W3r[:, YOFF[g]:YOFF[g] + SG_NCOL[g], :].transpose(1, 0, 2)
    w3g = np.ascontiguousarray(w3g.reshape(48, 9 * 5 * 144))

    in_maps = []
    for b in range(NB):
        connT = np.ascontiguousarray(conn[b].astype(f).T)
        vcat = np.concatenate([v[b].reshape(128, -1) for v in verts], axis=1)
        sph_cat = np.concatenate([s[b][:, :, 0, :] for s in sphs], axis=-1)
        in_maps.append(dict(connT=connT, vcat=np.ascontiguousarray(vcat),
                            sph=np.ascontiguousarray(sph_cat.reshape(128, 128 * 9)),
                            w2=w2p, w3g=w3g))

    res = run_bass_kernel_spmd(_get_nc(), in_maps, list(range(NB)))
    Z = np.stack([res.results[b]["zout"] for b in range(NB)])   # [8, 144, 128]

    # host epilogue: unpack e=(l,cp,k) rows, global per-l normalization
    out = np.zeros((NB, 128, 1, 16, 9), dtype=f)
    koff = [0, 1, 4]
    for l in range(3):
        blk = Z[:, FOFF[l]:FOFF[l] + 16 * LDIM[l], :]
        blk = blk.reshape(NB, 16, LDIM[l], 128).transpose(0, 3, 1, 2)
        nf = np.sum(blk.astype(np.float64) ** 2)
        out[:, :, 0, :, koff[l]:koff[l] + LDIM[l]] = blk / np.sqrt(nf / 16.0)
    return out



# revision 6
# speedup vs baseline: 2.9351x; 2.9351x over previous
"""Trainium2 Bass kernel for nn_CGLayer (gnn_message_passing).

Contract: kernel(**inputs) takes FULL inputs (as reference.setup_inputs()),
returns FULL output [8,128,1,16,9] f32. Internally: data-parallel over the
batch dim across 8 NeuronCores; per core one batch element.

Algebraic reduction (exact):
  X   = conn @ vertices                  (message passing, per batch)
  Y   = mix_nl(cg(X, X))                 (per-node quadratic in X)
  S   = sum_j sph[:, j, :]               (neighbor sum commutes through the
  Z   = mix_rel(cg(Y, S))                 relative-CG stage: x-side is
  out = Z / sqrt(sum Z^2 / 16)            j-independent)

Device pipeline per core, all-bf16 on the engines (PSUM accum f32):
  A:  X[i,144]    = matmul(lhsT=connT, rhs=vcat)            (bf16)
  S:  Ssum[i,9]   = reduce_j(sph);  S[9,i] = PE-transpose
  B:  P[i,9984]   = 13 DVE tensor_tensor ops (pair products, dense
                    q-major slot layout, l1==l2 m-pairs folded by symmetry)
      78 chunks:  PE transpose (bf16) -> PSUM -> copy to SBUF ->
                  matmul lhsT=W2-chunk, PSUM-accumulated per s-group g
  C:  per n=0..8: rep = sel9_n.T @ S (broadcast row), P2 = Y * rep (DVE),
      Z[i,144]   += P2_(g).T @ W3[n,g]   (45 accumulated matmuls)
Host epilogue: gather, unpack e=(l,c',k), global normalization per l.
"""
import numpy as np
from math import factorial, sqrt

MAXL = 2
CH = 16
NN = 128
NB = 8
LDIM = [1, 3, 5]
FOFF = [0, 16, 64]
NF = 144
TAU_NL = [768, 1536, 1536]
TAU_REL = [48, 96, 96]

# ------------------------------------------------------------- CG tables
def _cg_coeff(j1, m1, j2, m2, j3, m3):
    if m3 != m1 + m2:
        return 0.0
    pre = sqrt((2 * j3 + 1) * factorial(j3 + j1 - j2) * factorial(j3 - j1 + j2)
               * factorial(j1 + j2 - j3) / factorial(j1 + j2 + j3 + 1))
    pre *= sqrt(factorial(j3 + m3) * factorial(j3 - m3) * factorial(j1 - m1)
                * factorial(j1 + m1) * factorial(j2 - m2) * factorial(j2 + m2))
    s = 0.0
    vmin = max(0, j2 - j3 - m1, j1 - j3 + m2)
    vmax = min(j1 + j2 - j3, j1 - m1, j2 + m2)
    for v in range(vmin, vmax + 1):
        s += (-1) ** v / (factorial(v) * factorial(j1 + j2 - j3 - v)
                          * factorial(j1 - m1 - v) * factorial(j2 + m2 - v)
                          * factorial(j3 - j2 + m1 + v) * factorial(j3 - j1 - m2 + v))
    return pre * s


def _cg_matrix(l1, l2, l):
    M = np.zeros((2 * l1 + 1, 2 * l2 + 1, 2 * l + 1))
    for m1 in range(-l1, l1 + 1):
        for m2 in range(-l2, l2 + 1):
            if -l <= m1 + m2 <= l:
                M[m1 + l1, m2 + l2, m1 + m2 + l] = _cg_coeff(l1, m1, l2, m2, l, m1 + m2)
    return M


def _valid_pairs(l):
    return [(l1, l2) for l1 in range(3) for l2 in range(3)
            if abs(l1 - l2) <= l <= l1 + l2]

# ----------------------------------------------------- stage-B slot layout
# Dense q-major layout. A "q" is (l1, l2, m1) with l1 <= l2 and a contiguous
# m2 range after clipping |mt1+mt2| <= 2; for l1 == l2 additionally m2 >= m1
# (the symmetric product X x X makes (m1,m2)/(m2,m1) redundant up to a (c,d)
# transpose, folded into W2). Each (q, m2) block is 256 slots = (c,d) pairs.


def _build_q():
    qs = []
    for l1 in range(3):
        for l2 in range(l1, 3):
            for m1 in range(2 * l1 + 1):
                mt1 = m1 - l1
                m2_lo = max(0, -2 - mt1 + l2)
                m2_hi = min(2 * l2, 2 - mt1 + l2)
                if l1 == l2:
                    m2_lo = max(m2_lo, m1)
                if m2_lo > m2_hi:
                    continue
                qs.append((l1, l2, m1, m2_lo, m2_hi))
    return qs

QS = _build_q()                                          # 13 product ops
QOFF = np.concatenate([[0], np.cumsum([(q[4] - q[3] + 1) * 256 for q in QS])])
NSLOT = int(QOFF[-1])                                    # 9984
NCHUNK = NSLOT // 128                                    # 78

_DOFF = {}
for _k, (_l1, _l2, _m1, _lo, _hi) in enumerate(QS):
    for _m2 in range(_lo, _hi + 1):
        _DOFF[(_l1, _l2, _m1, _m2)] = int(QOFF[_k]) + (_m2 - _lo) * 256

# chunk -> s-group g (each 256-slot block is exactly 2 chunks)
_BLOCK_OF_CHUNK = []
for _k, (_l1, _l2, _m1, _lo, _hi) in enumerate(QS):
    for _m2 in range(_lo, _hi + 1):
        _BLOCK_OF_CHUNK += [(_l1, _l2, _m1, _m2)] * 2
G_OF_CHUNK = [(m1 - l1) + (m2 - l2) + 2 for (l1, l2, m1, m2) in _BLOCK_OF_CHUNK]
G_FIRST, G_LAST = {}, {}
for _ch, _g in enumerate(G_OF_CHUNK):
    G_FIRST.setdefault(_g, _ch)
    G_LAST[_g] = _ch


def _sgroup_cols(g):
    st = g - 2
    return [(l, cp) for l in range(3) if abs(st) <= l for cp in range(CH)]

SG_NCOL = [len(_sgroup_cols(g)) for g in range(5)]      # [16,32,48,32,16]
YOFF = np.concatenate([[0], np.cumsum(SG_NCOL)])        # Y piece row offsets


def _sg_lblock_col(g, l):
    st = g - 2
    return 16 * sum(1 for lp in range(l) if abs(st) <= lp)

_CAR, _DAR = np.meshgrid(np.arange(16), np.arange(16), indexing="ij")


def _assemble_W2(w_nl):
    """W2[NSLOT, 48] f64->f32: combined CG x w_nl, dense folded layout."""
    W2 = np.zeros((NSLOT, 48))
    for l in range(3):
        off = 0
        for (p1, p2) in _valid_pairs(l):
            Cg = _cg_matrix(p1, p2, l)
            wl = np.asarray(w_nl[l], np.float64)
            for m1 in range(2 * p1 + 1):
                for m2 in range(2 * p2 + 1):
                    st = (m1 - p1) + (m2 - p2)
                    if abs(st) > l:
                        continue
                    gc = Cg[m1, m2, st + l]
                    if gc == 0.0:
                        continue
                    g = st + 2
                    if p1 < p2 or (p1 == p2 and m1 <= m2):
                        slots = _DOFF[(p1, p2, m1, m2)] + _CAR * 16 + _DAR
                    else:
                        slots = _DOFF[(p2, p1, m2, m1)] + _DAR * 16 + _CAR
                    t = off + _CAR * 16 + _DAR
                    c0 = _sg_lblock_col(g, l)
                    W2[slots.ravel(), c0:c0 + 16] += gc * wl[t.ravel(), :]
            off += 256
    return W2


def _assemble_W3(w_rel):
    """W3[9*144, 144]: contraction P2[(n,a), i] -> Z[e, i]; a = Y piece row."""
    SOFF = [0, 1, 4]
    W3 = np.zeros((9 * 144, 144))
    ar = np.arange(16)
    for l in range(3):
        off = 0
        for (p1, p2) in _valid_pairs(l):
            Cg = _cg_matrix(p1, p2, l)
            wr = np.asarray(w_rel[l], np.float64)
            for m1 in range(2 * p1 + 1):
                for m2 in range(2 * p2 + 1):
                    st = (m1 - p1) + (m2 - p2)
                    if abs(st) > l:
                        continue
                    gc = Cg[m1, m2, st + l]
                    if gc == 0.0:
                        continue
                    gY = (m1 - p1) + 2
                    a0 = YOFF[gY] + _sg_lblock_col(gY, p1)
                    rows = (SOFF[p2] + m2) * 144 + a0 + ar
                    cols = FOFF[l] + (st + l) + ar * LDIM[l]
                    W3[np.ix_(rows, cols)] += gc * wr[off:off + 16, :]
            off += 16
    return W3

# ------------------------------------------------------------ bass builder
_NC_CACHE = {}


def _build_nc(debug=False):
    import concourse.bacc as bacc
    import concourse.bass as bass
    import concourse.tile as tile
    from concourse import mybir
    from concourse.masks import make_identity

    f32 = mybir.dt.float32
    bf16 = mybir.dt.bfloat16
    nc = bacc.Bacc()
    d_connT = nc.declare_dram_parameter("connT", [128, 128], bf16, isOutput=False)
    d_vcat = nc.declare_dram_parameter("vcat", [128, NF], bf16, isOutput=False)
    d_sph = nc.declare_dram_parameter("sph", [128, 128 * 9], bf16, isOutput=False)
    d_w2 = nc.declare_dram_parameter("w2", [128, NCHUNK * 48], bf16, isOutput=False)
    d_w3g = nc.declare_dram_parameter("w3g", [48, 9 * 5 * 144], bf16, isOutput=False)
    d_zout = nc.declare_dram_parameter("zout", [128, NF], f32, isOutput=True)
    if debug:
        d_dbgx = nc.declare_dram_parameter("dbgx", [128, NF], f32, isOutput=True)
        d_dbgs = nc.declare_dram_parameter("dbgs", [9, 128], f32, isOutput=True)
        d_dbgp = nc.declare_dram_parameter("dbgp", [128, NSLOT], f32, isOutput=True)
        d_dbgy = nc.declare_dram_parameter("dbgy", [48, 5 * 128], f32, isOutput=True)

    def vap(t, doff, freedims):
        base = t[:] if not isinstance(t, bass.AP) else t
        return bass.AP(tensor=base.tensor, offset=base.offset + doff,
                       ap=[list(base.ap[0])] + [list(d) for d in freedims])

    with tile.TileContext(nc) as tc:
      with (
        tc.tile_pool(name="big", bufs=1) as big,
        tc.tile_pool(name="sb", bufs=1) as sb,
        tc.tile_pool(name="pt", bufs=4) as ptp,
        tc.tile_pool(name="p2", bufs=4) as p2p,
        tc.tile_pool(name="ps_m", bufs=1, space="PSUM") as ps_m,
        tc.tile_pool(name="ps_t", bufs=3, space="PSUM") as ps_t,
        tc.tile_pool(name="ps_y", bufs=1, space="PSUM") as ps_y,
        tc.tile_pool(name="ps_r", bufs=1, space="PSUM") as ps_r,
      ):
        # ---- input DMAs (order matters: stage-A inputs first)
        connT = sb.tile([128, 128], bf16)
        nc.sync.dma_start(out=connT, in_=d_connT[:, :])
        vcat = sb.tile([128, NF], bf16)
        nc.sync.dma_start(out=vcat, in_=d_vcat[:, :])
        w2 = big.tile([128, NCHUNK, 48], bf16)
        for q in range(4):
            lo = (NCHUNK * q // 4) * 48
            hi = (NCHUNK * (q + 1) // 4) * 48
            nc.sync.dma_start(out=vap(w2, lo, [[1, hi - lo]]),
                              in_=d_w2[:, lo:hi])
        sph = big.tile([128, 128 * 9], bf16)
        nc.sync.dma_start(out=sph, in_=d_sph[:, :])
        w3g = sb.tile([48, 9, 5, 144], bf16)
        nc.sync.dma_start(
            out=w3g, in_=d_w3g[:, :].rearrange("p (n g e) -> p n g e", n=9, g=5))
        ident = sb.tile([128, 128], bf16)
        make_identity(nc, ident)

        # ---- stage A: X[i, feat] = connT.T @ vcat
        x_ps = ps_m.tile([128, NF], f32, tag="misc", name="x_ps")
        nc.tensor.matmul(x_ps, connT, vcat, start=True, stop=True)
        X = sb.tile([128, NF], bf16)
        nc.scalar.activation(X, x_ps, mybir.ActivationFunctionType.Copy)

        # ---- stage S: Ssum[i, 9] = sum_j sph; S[9, i]
        ssum = sb.tile([128, 9], f32)
        nc.vector.tensor_reduce(
            ssum, vap(sph, 0, [[1, 9], [9, 128]]),
            mybir.AxisListType.X, mybir.AluOpType.add)
        ssum_bf = sb.tile([128, 9], bf16)
        nc.gpsimd.tensor_copy(out=ssum_bf, in_=ssum)
        s_ps = ps_m.tile([9, 128], bf16, name="s_ps")
        nc.tensor.transpose(s_ps, ssum_bf, ident)
        S = sb.tile([9, 128], bf16)
        nc.scalar.activation(S, s_ps, mybir.ActivationFunctionType.Copy)
        sel9 = sb.tile([9, 9, 128], bf16)
        nc.gpsimd.memset(sel9, 0.0)
        nc.gpsimd.affine_select(
            out=sel9, in_=sel9, compare_op=mybir.AluOpType.not_equal,
            fill=1.0, base=0, pattern=[[-1, 9], [0, 128]], channel_multiplier=1)

        # ---- stage B products: P[i, NSLOT] (dense, no memset needed)
        P = big.tile([128, NSLOT], bf16)
        for k, (l1, l2, m1, m2_lo, m2_hi) in enumerate(QS):
            nm2 = m2_hi - m2_lo + 1
            nc.vector.tensor_tensor(
                out=vap(P, int(QOFF[k]), [[256, nm2], [16, 16], [1, 16]]),
                in0=vap(X, FOFF[l1] + m1, [[0, nm2], [LDIM[l1], 16], [0, 16]]),
                in1=vap(X, FOFF[l2] + m2_lo,
                        [[1, nm2], [0, 16], [LDIM[l2], 16]]),
                op=mybir.AluOpType.mult)

        # ---- stage B transposes + mix: Y pieces, PSUM-accumulated per g.
        # Chunks iterate g-major: accumulation groups sharing a PSUM bank
        # must be sequential (an interleaved start=True clobbers other open
        # groups in the same bank).
        ymix = ps_y.tile([48, 5, 128], f32)
        chunk_order = sorted(range(NCHUNK), key=lambda c: (G_OF_CHUNK[c], c))
        for i, ch in enumerate(chunk_order):
            g = G_OF_CHUNK[ch]
            ncol = SG_NCOL[g]
            t_ps = ps_t.tile([128, 128], bf16)
            nc.tensor.transpose(t_ps, P[:, ch * 128:(ch + 1) * 128], ident)
            pt = ptp.tile([128, 128], bf16)
            if i % 2 == 0:
                nc.scalar.activation(pt, t_ps, mybir.ActivationFunctionType.Copy)
            else:
                nc.vector.tensor_copy(out=pt, in_=t_ps)
            nc.tensor.matmul(ymix[0:ncol, g, :], w2[:, ch, 0:ncol], pt,
                             start=(i == 0 or G_OF_CHUNK[chunk_order[i - 1]] != g),
                             stop=(i == NCHUNK - 1 or G_OF_CHUNK[chunk_order[i + 1]] != g))

        # Y pieces -> SBUF, packed [48, 5, 128] bf16
        ysb = sb.tile([48, 5, 128], bf16)
        nc.scalar.activation(ysb, ymix, mybir.ActivationFunctionType.Copy)

        if debug:
            xdb = sb.tile([128, NF], f32)
            nc.vector.tensor_copy(out=xdb, in_=X)
            nc.sync.dma_start(out=d_dbgx[:, :], in_=xdb)
            sdb = sb.tile([9, 128], f32)
            nc.vector.tensor_copy(out=sdb, in_=S)
            nc.sync.dma_start(out=d_dbgs[:, :], in_=sdb)
            pdb = big.tile([128, NSLOT], f32)
            nc.vector.tensor_copy(out=pdb, in_=P)
            for q in range(4):
                lo = NSLOT * q // 4
                hi = NSLOT * (q + 1) // 4
                nc.sync.dma_start(out=d_dbgp[:, lo:hi],
                                  in_=vap(pdb, lo, [[1, hi - lo]]))
            ydb = sb.tile([48, 5 * 128], f32)
            nc.vector.tensor_copy(out=ydb, in_=vap(ysb, 0, [[1, 5 * 128]]))
            nc.sync.dma_start(out=d_dbgy[:, :], in_=ydb)

        # ---- stage C: P2 = Y * rep(S_n); Z[i, e] = sum_{n,g} P2_g.T @ W3[n,g]
        zps = ps_m.tile([128, NF], f32, tag="misc", name="z_ps")
        for n in range(9):
            rep = ps_r.tile([48, 128], f32)
            nc.tensor.matmul(rep, sel9[:, n, 0:48], S, start=True, stop=True)
            p2 = p2p.tile([48, 5, 128], bf16)
            nc.vector.tensor_tensor(
                out=p2,
                in0=vap(rep, 0, [[0, 5], [1, 128]]),
                in1=ysb, op=mybir.AluOpType.mult)
            for g in range(5):
                ncol = SG_NCOL[g]
                nc.tensor.matmul(zps, p2[0:ncol, g, :], w3g[0:ncol, n, g, :],
                                 start=(n == 0 and g == 0), stop=(n == 8 and g == 4))

        zs = sb.tile([128, NF], f32)
        nc.scalar.activation(zs, zps, mybir.ActivationFunctionType.Copy)
        nc.sync.dma_start(out=d_zout[:, :], in_=zs)

    nc.compile()
    return nc

# ------------------------------------------------------------- host entry
def _get_nc(debug=False):
    key = ("dbg" if debug else "nc")
    if key not in _NC_CACHE:
        _NC_CACHE[key] = _build_nc(debug)
    return _NC_CACHE[key]


def kernel(vertices_0, vertices_1, vertices_2, connectivity,
           sph_0, sph_1, sph_2,
           w_nl_0, w_nl_1, w_nl_2,
           w_rel_0, w_rel_1, w_rel_2, _debug=False):
    from concourse.bass_utils import run_bass_kernel_spmd
    import ml_dtypes

    f = np.float32
    bf = ml_dtypes.bfloat16
    verts = [np.asarray(v, f) for v in (vertices_0, vertices_1, vertices_2)]
    sphs = [np.asarray(s, f) for s in (sph_0, sph_1, sph_2)]
    conn = np.asarray(connectivity)
    W2 = _assemble_W2([np.asarray(w, f) for w in (w_nl_0, w_nl_1, w_nl_2)])
    W3 = _assemble_W3([np.asarray(w, f) for w in (w_rel_0, w_rel_1, w_rel_2)])
    # pack to SBUF-ready layouts (shared across cores)
    w2p = np.ascontiguousarray(
        W2.reshape(NCHUNK, 128, 48).transpose(1, 0, 2).reshape(128, NCHUNK * 48)
    ).astype(bf)
    W3r = W3.reshape(9, 144, 144)
    w3g = np.zeros((48, 9, 5, 144), np.float64)
    for g in range(5):
        w3g[0:SG_NCOL[g], :, g, :] = W3r[:, YOFF[g]:YOFF[g] + SG_NCOL[g], :].transpose(1, 0, 2)
    w3g = np.ascontiguousarray(w3g.reshape(48, 9 * 5 * 144)).astype(bf)

    in_maps = []
    for b in range(NB):
        connT = np.ascontiguousarray(conn[b].astype(f).T).astype(bf)
        vcat = np.concatenate([v[b].reshape(128, -1) for v in verts], axis=1)
        sph_cat = np.concatenate([s[b][:, :, 0, :] for s in sphs], axis=-1)
        in_maps.append(dict(connT=connT, vcat=np.ascontiguousarray(vcat).astype(bf),
                            sph=np.ascontiguousarray(
                                sph_cat.reshape(128, 128 * 9)).astype(bf),
                            w2=w2p, w3g=w3g))

    res = run_bass_kernel_spmd(_get_nc(_debug), in_maps, list(range(NB)))
    if _debug:
        kernel._dbg = res
    Z = np.stack([res.results[b]["zout"] for b in range(NB)])   # [8, 128, 144]

    # host epilogue: unpack e=(l,cp,k) cols, global per-l normalization
    out = np.zeros((NB, 128, 1, 16, 9), dtype=f)
    koff = [0, 1, 4]
    for l in range(3):
        blk = Z[:, :, FOFF[l]:FOFF[l] + 16 * LDIM[l]]
        blk = blk.reshape(NB, 128, 16, LDIM[l])
        nf = np.sum(blk.astype(np.float64) ** 2)
        out[:, :, 0, :, koff[l]:koff[l] + LDIM[l]] = blk / np.sqrt(nf / 16.0)
    return out


# revision 23
# speedup vs baseline: 4.3067x; 1.4673x over previous
"""Trainium2 Bass kernel for nn_CGLayer (gnn_message_passing).

Contract: kernel(**inputs) takes FULL inputs (as reference.setup_inputs()),
returns FULL output [8,128,1,16,9] f32. Internally: data-parallel over the
batch dim across 8 NeuronCores; per core one batch element.

Algebraic reduction (exact):
  X   = conn @ vertices                  (message passing, per batch)
  Y   = mix_nl(cg(X, X))                 (per-node quadratic in X)
  S   = sum_j sph[:, j, :]               (neighbor sum commutes through the
  Z   = mix_rel(cg(Y, S))                 relative-CG stage: x-side is
  out = Z / sqrt(sum Z^2 / 16)            j-independent)

Device pipeline per core (bf16 engines, f32 PSUM accumulate):
  A:  X[i,144]    = matmul(lhsT=connT, rhs=vcat)
  B:  P[i,9984]   = 13 tensor_tensor product ops split DVE/GpSimd (dense
                    q-major slot layout, l1==l2 m-pairs folded by symmetry)
      PT[s,ch,i]  = XBAR dma_start_transpose of P (no PE, no PSUM copies)
      Y           = 78 matmuls lhsT=W2-chunk rhs=PT-chunk, PSUM-accumulated
                    g-major (interleaved PSUM accumulation groups sharing a
                    bank break; sequential groups are fine)
  S:  Ssum=reduce_j(sph) -> XBAR transpose -> DRAM -> broadcast-read SREP
  C:  P2[a,n,g,i] = SREP * Y (DVE 2x);  Z[i,144] += P2_(n,g).T @ W3[n,g]
Host epilogue: gather, unpack e=(l,c',k), global normalization per l.
"""
import numpy as np
from math import factorial, sqrt

MAXL = 2
CH = 16
NN = 128
NB = 8
LDIM = [1, 3, 5]
FOFF = [0, 16, 64]
NF = 144

# ------------------------------------------------------------- CG tables
def _cg_coeff(j1, m1, j2, m2, j3, m3):
    if m3 != m1 + m2:
        return 0.0
    pre = sqrt((2 * j3 + 1) * factorial(j3 + j1 - j2) * factorial(j3 - j1 + j2)
               * factorial(j1 + j2 - j3) / factorial(j1 + j2 + j3 + 1))
    pre *= sqrt(factorial(j3 + m3) * factorial(j3 - m3) * factorial(j1 - m1)
                * factorial(j1 + m1) * factorial(j2 - m2) * factorial(j2 + m2))
    s = 0.0
    vmin = max(0, j2 - j3 - m1, j1 - j3 + m2)
    vmax = min(j1 + j2 - j3, j1 - m1, j2 + m2)
    for v in range(vmin, vmax + 1):
        s += (-1) ** v / (factorial(v) * factorial(j1 + j2 - j3 - v)
                          * factorial(j1 - m1 - v) * factorial(j2 + m2 - v)
                          * factorial(j3 - j2 + m1 + v) * factorial(j3 - j1 - m2 + v))
    return pre * s


def _cg_matrix(l1, l2, l):
    M = np.zeros((2 * l1 + 1, 2 * l2 + 1, 2 * l + 1))
    for m1 in range(-l1, l1 + 1):
        for m2 in range(-l2, l2 + 1):
            if -l <= m1 + m2 <= l:
                M[m1 + l1, m2 + l2, m1 + m2 + l] = _cg_coeff(l1, m1, l2, m2, l, m1 + m2)
    return M


def _valid_pairs(l):
    return [(l1, l2) for l1 in range(3) for l2 in range(3)
            if abs(l1 - l2) <= l <= l1 + l2]

# ----------------------------------------------------- stage-B slot layout
# Dense q-major layout. A "q" is (l1, l2, m1) with l1 <= l2 and a contiguous
# m2 range after clipping |mt1+mt2| <= 2; for l1 == l2 additionally m2 >= m1
# (the symmetric product X x X makes (m1,m2)/(m2,m1) redundant up to a (c,d)
# transpose, folded into W2). Each (q, m2) block is 256 slots = (c,d) pairs.


def _build_q():
    qs = []
    for l1 in range(3):
        for l2 in range(l1, 3):
            for m1 in range(2 * l1 + 1):
                mt1 = m1 - l1
                m2_lo = max(0, -2 - mt1 + l2)
                m2_hi = min(2 * l2, 2 - mt1 + l2)
                if l1 == l2:
                    m2_lo = max(m2_lo, m1)
                if m2_lo > m2_hi:
                    continue
                qs.append((l1, l2, m1, m2_lo, m2_hi))
    return qs

QS = _build_q()                                          # 13 product ops
QOFF = np.concatenate([[0], np.cumsum([(q[4] - q[3] + 1) * 256 for q in QS])])
NSLOT = int(QOFF[-1])                                    # 9984
NCHUNK = NSLOT // 128                                    # 78

_DOFF = {}
_BLOCK_OF_CHUNK = []
CH_Q = []
for _k, (_l1, _l2, _m1, _lo, _hi) in enumerate(QS):
    for _m2 in range(_lo, _hi + 1):
        _DOFF[(_l1, _l2, _m1, _m2)] = int(QOFF[_k]) + (_m2 - _lo) * 256
        _BLOCK_OF_CHUNK += [(_l1, _l2, _m1, _m2)] * 2
        CH_Q += [_k, _k]
G_OF_CHUNK = [(m1 - l1) + (m2 - l2) + 2 for (l1, l2, m1, m2) in _BLOCK_OF_CHUNK]

# product emission order: ops feeding low-g chunks first (mixes run g-major)
def _gmin(k):
    l1, l2, m1, lo, hi = QS[k]
    return min((m1 - l1) + (m2 - l2) + 2 for m2 in range(lo, hi + 1))

PROD_ORDER = sorted(range(len(QS)), key=lambda k: (_gmin(k), -(QS[k][4] - QS[k][3] + 1)))
_PRANK = {k: j for j, k in enumerate(PROD_ORDER)}

# engine split (0=DVE, 1=GpSimd), greedy balance in emission order
PROD_ENG = {}
_tD = _tP = 0.0
for _k in PROD_ORDER:
    _cols = (QS[_k][4] - QS[_k][3] + 1) * 256
    _cD = _tD + 150 + _cols * 1.04
    _cP = _tP + 300 + _cols * 1.00
    if _cD <= _cP:
        PROD_ENG[_k] = 0
        _tD = _cD
    else:
        PROD_ENG[_k] = 1
        _tP = _cP

# mix order: g-major; within g by product completion rank
MIX_ORDER = sorted(range(NCHUNK), key=lambda c: (G_OF_CHUNK[c], _PRANK[CH_Q[c]], c))


def _sgroup_cols(g):
    st = g - 2
    return [(l, cp) for l in range(3) if abs(st) <= l for cp in range(CH)]

SG_NCOL = [len(_sgroup_cols(g)) for g in range(5)]      # [16,32,48,32,16]
YOFF = np.concatenate([[0], np.cumsum(SG_NCOL)])        # Y piece row offsets


def _sg_lblock_col(g, l):
    st = g - 2
    return 16 * sum(1 for lp in range(l) if abs(st) <= lp)

_CAR, _DAR = np.meshgrid(np.arange(16), np.arange(16), indexing="ij")


def _assemble_W2(w_nl):
    """W2[NSLOT, 48] f64: combined CG x w_nl, dense folded layout."""
    W2 = np.zeros((NSLOT, 48))
    for l in range(3):
        off = 0
        for (p1, p2) in _valid_pairs(l):
            Cg = _cg_matrix(p1, p2, l)
            wl = np.asarray(w_nl[l], np.float64)
            for m1 in range(2 * p1 + 1):
                for m2 in range(2 * p2 + 1):
                    st = (m1 - p1) + (m2 - p2)
                    if abs(st) > l:
                        continue
                    gc = Cg[m1, m2, st + l]
                    if gc == 0.0:
                        continue
                    g = st + 2
                    if p1 < p2 or (p1 == p2 and m1 <= m2):
                        slots = _DOFF[(p1, p2, m1, m2)] + _CAR * 16 + _DAR
                    else:
                        slots = _DOFF[(p2, p1, m2, m1)] + _DAR * 16 + _CAR
                    t = off + _CAR * 16 + _DAR
                    c0 = _sg_lblock_col(g, l)
                    W2[slots.ravel(), c0:c0 + 16] += gc * wl[t.ravel(), :]
            off += 256
    return W2


def _assemble_W3(w_rel):
    """W3[9*144, 144]: contraction P2[(n,a), i] -> Z[e, i]; a = Y piece row."""
    SOFF = [0, 1, 4]
    W3 = np.zeros((9 * 144, 144))
    ar = np.arange(16)
    for l in range(3):
        off = 0
        for (p1, p2) in _valid_pairs(l):
            Cg = _cg_matrix(p1, p2, l)
            wr = np.asarray(w_rel[l], np.float64)
            for m1 in range(2 * p1 + 1):
                for m2 in range(2 * p2 + 1):
                    st = (m1 - p1) + (m2 - p2)
                    if abs(st) > l:
                        continue
                    gc = Cg[m1, m2, st + l]
                    if gc == 0.0:
                        continue
                    gY = (m1 - p1) + 2
                    a0 = YOFF[gY] + _sg_lblock_col(gY, p1)
                    rows = (SOFF[p2] + m2) * 144 + a0 + ar
                    cols = FOFF[l] + (st + l) + ar * LDIM[l]
                    W3[np.ix_(rows, cols)] += gc * wr[off:off + 16, :]
            off += 16
    return W3

# ------------------------------------------------------------ bass builder
_NC_CACHE = {}


def _build_nc(debug=False):
    import concourse.bacc as bacc
    import concourse.bass as bass
    import concourse.tile as tile
    from concourse import mybir
    from concourse.tile import add_dep_helper

    def dep(a, b, why):
        # annotate_deps misses InstDmaTransposeAnt / hand-built AP operands;
        # wire the edge explicitly.
        add_dep_helper(a.ins, b.ins, reason=why)

    f32 = mybir.dt.float32
    bf16 = mybir.dt.bfloat16
    nc = bacc.Bacc()
    d_cv = nc.declare_dram_parameter("cvcat", [128, 128 + NF], bf16, isOutput=False)
    d_sph = nc.declare_dram_parameter("sph", [128, 128 * 9], bf16, isOutput=False)
    d_w2 = nc.declare_dram_parameter("w2", [128, NCHUNK * 48], bf16, isOutput=False)
    d_w3g = nc.declare_dram_parameter("w3g", [48, 9 * 5 * 144], bf16, isOutput=False)
    d_zout = nc.declare_dram_parameter("zout", [128, NF], f32, isOutput=True)
    if debug:
        d_dbgx = nc.declare_dram_parameter("dbgx", [128, NF], f32, isOutput=True)
        d_dbgs = nc.declare_dram_parameter("dbgs", [48, 9 * 128], f32, isOutput=True)
        d_dbgp = nc.declare_dram_parameter("dbgp", [128, NSLOT], f32, isOutput=True)
        d_dbgy = nc.declare_dram_parameter("dbgy", [48, 5 * 128], f32, isOutput=True)
        d_dbgt = nc.declare_dram_parameter("dbgt", [128, NCHUNK * 128], f32, isOutput=True)

    def vap(t, doff, freedims):
        base = t[:] if not isinstance(t, bass.AP) else t
        return bass.AP(tensor=base.tensor, offset=base.offset + doff,
                       ap=[list(base.ap[0])] + [list(d) for d in freedims])

    with tile.TileContext(nc) as tc:
      with (
        tc.tile_pool(name="big", bufs=1) as big,
        tc.tile_pool(name="sb", bufs=1) as sb,
        tc.tile_pool(name="dr", bufs=1, space="DRAM") as drp,
        tc.tile_pool(name="ps_m", bufs=1, space="PSUM") as ps_m,
        tc.tile_pool(name="ps_y", bufs=1, space="PSUM") as ps_y,
      ):
        # ---- input DMAs (SP queue; stage-A inputs first)
        cv = sb.tile([128, 128 + NF], bf16)
        nc.sync.dma_start(out=cv, in_=d_cv[:, :])
        w2 = big.tile([128, NCHUNK, 48], bf16)
        for q in range(2):
            lo = (NCHUNK * q // 2) * 48
            hi = (NCHUNK * (q + 1) // 2) * 48
            nc.sync.dma_start(out=vap(w2, lo, [[1, hi - lo]]), in_=d_w2[:, lo:hi])
        sph = big.tile([128, 128 * 9], bf16)
        nc.sync.dma_start(out=sph, in_=d_sph[:, :])
        w3g = sb.tile([48, 9, 5, 144], bf16)
        nc.sync.dma_start(
            out=w3g, in_=d_w3g[:, :].rearrange("p (n g e) -> p n g e", n=9, g=5))

        # ---- stage A: X[i, feat] = connT.T @ vcat  (ACT copies PSUM->bf16)
        x_ps = ps_m.tile([128, NF], f32, tag="misc", name="x_ps")
        nc.tensor.matmul(x_ps, cv[:, 0:128], cv[:, 128:128 + NF],
                         start=True, stop=True)
        X = sb.tile([128, NF], bf16)
        nc.scalar.activation(X, x_ps, mybir.ActivationFunctionType.Copy)

        # ---- stage B products: P[i, NSLOT], split DVE / GpSimd
        P = big.tile([128, NSLOT], bf16)
        pinst = {}
        for k in PROD_ORDER:
            l1, l2, m1, m2_lo, m2_hi = QS[k]
            nm2 = m2_hi - m2_lo + 1
            eng = nc.vector if PROD_ENG[k] == 0 else nc.gpsimd
            pinst[k] = eng.tensor_tensor(
                out=vap(P, int(QOFF[k]), [[256, nm2], [16, 16], [1, 16]]),
                in0=vap(X, FOFF[l1] + m1, [[0, nm2], [LDIM[l1], 16], [0, 16]]),
                in1=vap(X, FOFF[l2] + m2_lo,
                        [[1, nm2], [0, 16], [LDIM[l2], 16]]),
                op=mybir.AluOpType.mult)

        # ---- XBAR transposes: PT[s, ch, i] = P[i, ch*128+s]
        PT = big.tile([128, NCHUNK, 128], bf16)
        tinst = {}
        # All on one queue: concurrent XBAR transposes from two queues
        # corrupt each other. Drain fences make consumers wait for DMA
        # *completion* (a plain dep edge only orders against issue).
        fence_at = {5: 0, 7: 1, 10: 2, 12: 3}   # PROD_ORDER index -> fence id
        fences = {}
        for j, k in enumerate(PROD_ORDER):
            lo, hi = int(QOFF[k]), int(QOFF[k + 1])
            tinst[k] = nc.sync.dma_start_transpose(out=PT[:, lo // 128:hi // 128, :],
                                                   in_=P[:, lo:hi])
            dep(tinst[k], pinst[k], "xbar reads P block")
            if j in fence_at:
                fences[fence_at[j]] = nc.sync.drain()
        # fence id covering each g's full set of feeding transposes
        G_FENCE = {0: 0, 1: 1, 2: 2, 3: 2, 4: 3}

        # ---- stage S: Ssum -> XBAR transpose -> DRAM -> broadcast SREP
        # (DVE reduce emitted after the products; it fills the gap while the
        # mixes finish. GpSimd only does cross-partition reduces.)
        ssum = sb.tile([128, 9], f32)
        nc.vector.tensor_reduce(
            ssum, vap(sph, 0, [[1, 9], [9, 128]]),
            mybir.AxisListType.X, mybir.AluOpType.add)
        sp_pad = sb.tile([128, 128], bf16)
        spc = nc.gpsimd.tensor_copy(out=sp_pad[:, 0:9], in_=ssum)
        spm = nc.gpsimd.memset(sp_pad[:, 9:128], 0.0)
        S128 = sb.tile([128, 128], bf16)
        s128t = nc.sync.dma_start_transpose(out=S128, in_=sp_pad)
        dep(s128t, spc, "S xbar reads sp_pad")
        dep(s128t, spm, "S xbar reads sp_pad pad")
        stmp = drp.tile([9, 128], bf16)
        stw = nc.sync.dma_start(out=stmp[:, :], in_=S128[0:9, :])
        dep(stw, s128t, "stmp reads S128")
        SREP = sb.tile([48, 9, 128], bf16)
        srr = nc.sync.dma_start(
            out=SREP,
            in_=bass.AP(tensor=stmp[:].tensor, offset=stmp[:].offset,
                        ap=[[0, 48], [128, 9], [1, 128]]))
        dep(srr, stw, "SREP reads stmp dram")
        sfence = nc.sync.drain()

        # ---- stage B mixes: Y pieces, PSUM-accumulated g-major
        ymix = ps_y.tile([48, 5, 128], f32)
        for i, ch in enumerate(MIX_ORDER):
            g = G_OF_CHUNK[ch]
            ncol = SG_NCOL[g]
            mm = nc.tensor.matmul(
                ymix[0:ncol, g, :], w2[:, i, 0:ncol], PT[:, ch, :],
                start=(i == 0 or G_OF_CHUNK[MIX_ORDER[i - 1]] != g),
                stop=(i == NCHUNK - 1 or G_OF_CHUNK[MIX_ORDER[i + 1]] != g))
            dep(mm, fences[G_FENCE[g]], "mix waits PT xbar drain")

        ysb = sb.tile([48, 5, 128], bf16)
        nc.scalar.activation(ysb, ymix, mybir.ActivationFunctionType.Copy)

        if debug:
            xdb = sb.tile([128, NF], f32)
            nc.vector.tensor_copy(out=xdb, in_=X)
            nc.sync.dma_start(out=d_dbgx[:, :], in_=xdb)
            sdb = sb.tile([48, 9 * 128], f32)
            sdbc = nc.vector.tensor_copy(out=sdb, in_=vap(SREP, 0, [[1, 9 * 128]]))
            dep(sdbc, srr, "dbg reads SREP")
            nc.sync.dma_start(out=d_dbgs[:, :], in_=sdb)
            pdb = big.tile([128, NSLOT], f32)
            nc.vector.tensor_copy(out=pdb, in_=P)
            for q in range(4):
                lo = NSLOT * q // 4
                hi = NSLOT * (q + 1) // 4
                nc.sync.dma_start(out=d_dbgp[:, lo:hi],
                                  in_=vap(pdb, lo, [[1, hi - lo]]))
            ydb = sb.tile([48, 5 * 128], f32)
            nc.vector.tensor_copy(out=ydb, in_=vap(ysb, 0, [[1, 5 * 128]]))
            nc.sync.dma_start(out=d_dbgy[:, :], in_=ydb)
            tdb = big.tile([128, NCHUNK * 128], f32)
            tdbc = nc.vector.tensor_copy(out=tdb, in_=vap(PT, 0, [[1, NCHUNK * 128]]))
            for k in tinst:
                dep(tdbc, tinst[k], "dbg reads PT")
            for q in range(8):
                lo = NCHUNK * 128 * q // 8
                hi = NCHUNK * 128 * (q + 1) // 8
                nc.sync.dma_start(out=d_dbgt[:, lo:hi],
                                  in_=vap(tdb, lo, [[1, hi - lo]]))

        # ---- stage C: P2 = SREP * Y (DVE 2x); Z[i,e] = sum P2_(n,g).T @ W3
        p2 = sb.tile([48, 9, 5, 128], bf16)
        for n in range(9):
            p2i = nc.vector.tensor_tensor(
                out=p2[:, n, :, :],
                in0=vap(SREP, n * 128, [[0, 5], [1, 128]]),
                in1=ysb, op=mybir.AluOpType.mult)
            dep(p2i, sfence, "p2 waits SREP dma drain")
        zps = ps_m.tile([128, NF], f32, tag="misc", name="z_ps")
        for n in range(9):
            for g in range(5):
                ncol = SG_NCOL[g]
                nc.tensor.matmul(zps, p2[0:ncol, n, g, :], w3g[0:ncol, n, g, :],
                                 start=(n == 0 and g == 0), stop=(n == 8 and g == 4))

        zs = sb.tile([128, NF], f32)
        nc.scalar.activation(zs, zps, mybir.ActivationFunctionType.Copy)
        nc.sync.dma_start(out=d_zout[:, :], in_=zs)

    nc.compile()
    return nc

# ------------------------------------------------------------- host entry
def _get_nc(debug=False):
    key = ("dbg" if debug else "nc")
    if key not in _NC_CACHE:
        _NC_CACHE[key] = _build_nc(debug)
    return _NC_CACHE[key]


def kernel(vertices_0, vertices_1, vertices_2, connectivity,
           sph_0, sph_1, sph_2,
           w_nl_0, w_nl_1, w_nl_2,
           w_rel_0, w_rel_1, w_rel_2, _debug=False):
    from concourse.bass_utils import run_bass_kernel_spmd
    import ml_dtypes

    f = np.float32
    bf = ml_dtypes.bfloat16
    verts = [np.asarray(v, f) for v in (vertices_0, vertices_1, vertices_2)]
    sphs = [np.asarray(s, f) for s in (sph_0, sph_1, sph_2)]
    conn = np.asarray(connectivity)
    W2 = _assemble_W2([np.asarray(w, f) for w in (w_nl_0, w_nl_1, w_nl_2)])
    W3 = _assemble_W3([np.asarray(w, f) for w in (w_rel_0, w_rel_1, w_rel_2)])
    # pack to SBUF-ready layouts (shared across cores); w2 chunks in MIX_ORDER
    w2p = np.ascontiguousarray(
        W2.reshape(NCHUNK, 128, 48)[MIX_ORDER].transpose(1, 0, 2)
        .reshape(128, NCHUNK * 48)).astype(bf)
    W3r = W3.reshape(9, 144, 144)
    w3g = np.zeros((48, 9, 5, 144), np.float64)
    for g in range(5):
        w3g[0:SG_NCOL[g], :, g, :] = W3r[:, YOFF[g]:YOFF[g] + SG_NCOL[g], :].transpose(1, 0, 2)
    w3g = np.ascontiguousarray(w3g.reshape(48, 9 * 5 * 144)).astype(bf)

    in_maps = []
    for b in range(NB):
        connT = np.ascontiguousarray(conn[b].astype(f).T)
        vcat = np.concatenate([v[b].reshape(128, -1) for v in verts], axis=1)
        cvcat = np.concatenate([connT, vcat], axis=1).astype(bf)
        sph_cat = np.concatenate([s[b][:, :, 0, :] for s in sphs], axis=-1)
        in_maps.append(dict(cvcat=np.ascontiguousarray(cvcat),
                            sph=np.ascontiguousarray(
                                sph_cat.reshape(128, 128 * 9)).astype(bf),
                            w2=w2p, w3g=w3g))

    res = run_bass_kernel_spmd(_get_nc(_debug), in_maps, list(range(NB)))
    if _debug:
        kernel._dbg = res
    Z = np.stack([res.results[b]["zout"] for b in range(NB)])   # [8, 128, 144]

    # host epilogue: unpack e=(l,cp,k) cols, global per-l normalization
    out = np.zeros((NB, 128, 1, 16, 9), dtype=f)
    koff = [0, 1, 4]
    for l in range(3):
        blk = Z[:, :, FOFF[l]:FOFF[l] + 16 * LDIM[l]]
        blk = blk.reshape(NB, 128, 16, LDIM[l])
        nf = np.sum(blk.astype(np.float64) ** 2)
        out[:, :, 0, :, koff[l]:koff[l] + LDIM[l]] = blk / np.sqrt(nf / 16.0)
    return out
